# revision 28
# baseline (speedup 1.0000x reference)
"""Trainium2 Bass kernel for nn_AC_13907104104944 (ragged_sequence).

Math: reference builds a [T, L] mask where token t covers frames
[s_t, ndc_t) with ndc_t <= s_{t+2}. Hence each output frame l is covered only
by a short consecutive token range. We shard the frame axis L across the
8 cores (sequence parallel), tile each core's frames into 128 windows of 16
frames (one window per SBUF partition), and give each window the consecutive
token range that can possibly cover it (conservative integer bound from the
duration cumsum -- host-side index bookkeeping only). All float math of the
reference (rerelu chains, new_dur, cumsum offsets, masked accumulation,
rerelu4, channel broadcast) runs on-device as dense [128, *] vector/scalar
engine ops. No collectives needed: cores' frame ranges are disjoint.

mel_pre[l] = sum_k a_k * (l+1-s_k) * [s_k <= l] * [l < ndc_k]   (a=rerelu3(tok)^2)
mel[l]     = rerelu4(mel_pre[l]),  out[0, l, :] = mel[l]  (128 channels)
"""

import numpy as np

import concourse.bass as bass
import concourse.mybir as mybir
from concourse.tile import TileContext
from concourse.bass_utils import run_bass_kernel_spmd

N_CORES = 8
WIN = 16          # frames per window (one window per partition)
NWIN = N_CORES * 128
F32 = mybir.dt.float32
ALU = mybir.AluOpType
ACTF = mybir.ActivationFunctionType

# rerelu as 12 clamp pairs: sum_k c_k * clamp(x - b_k, 0, 0.5)
PAIRS1 = [(3.25, 4.0), (5.25, -4.0), (6.25, 4.0), (11.25, -2.0), (15.25, 2.0),
          (20.25, -2.0), (23.25, 2.0), (35.25, -2.0), (40.25, 2.0),
          (43.25, -2.0), (49.25, 2.0), (61.25, -2.0)]
# rerelu2 - 1 as 3 clamp pairs on sum_token
PAIRS2 = [(1.25, -2.0), (2.25, 2.0), (3.25, -2.0)]


def _vw(ap, dims):
    """Replace the free dims of an AP (keep partition dim). dims: [[step, count],...]"""
    return bass.AP(ap.tensor, ap.offset, [list(ap.ap[0])] + [list(d) for d in dims])


def _fix_drain_waits(nc):
    """Walrus allows few sync-wait commands per instruction (1 for the CTRL
    drain, 2 for compute/DMA). Tile's final drain waits on the raw global
    clock without per-engine elision, so it can exceed that. Fix: re-home the
    drain's extra waits onto earlier SP instructions with spare wait slots
    (sound: the wait still executes before the drain in the same engine
    stream, and its producer was issued earlier), then strip waits that are
    dominated by earlier same-engine waits."""
    blocks = list(nc.m.functions[0].blocks)
    end_blk = blocks[-1]
    insts = list(end_blk.instructions)

    def waits(inst):
        si = inst.sync_info
        return list(si.on_wait) if si is not None and si.on_wait else []

    # Cross-engine dominated-wait tracking over the whole program. Sound for
    # the end-block drain only: every engine passes the all-engine barrier
    # after its drain and before the sem clears, so any sem value some engine
    # already waited for is reached before kernel end / sem reuse.
    seen = {}
    for blk in blocks[:-1]:
        for inst in blk.instructions:
            for w in waits(inst):
                if w.wait_mode == 'sem-ge-imm':
                    seen[w.ant_name] = max(seen.get(w.ant_name, -1),
                                           w.wait_value)

    drain = insts[0]
    dw = waits(drain)
    if type(drain).__name__ != 'InstDrain' or len(dw) <= 1:
        return
    remaining = [w for w in dw
                 if not (w.wait_mode == 'sem-ge-imm'
                         and seen.get(w.ant_name, -1) >= w.wait_value)]
    # hosts: later zero-wait drains in the end block (before the sem clears,
    # each engine passes the all-engine barrier afterwards)
    hosts = [i for i in insts[1:]
             if type(i).__name__ == 'InstDrain' and not waits(i)]
    keep = remaining[:1]
    excess = remaining[1:]
    assert len(excess) <= len(hosts), (
        f"not enough wait hosts: need {len(excess)}, have {len(hosts)}")
    import bass_rust
    for w, h in zip(excess, hosts):
        if h.sync_info is None:
            h.sync_info = bass_rust.SyncInfo(on_wait=[w], on_update=[])
        else:
            h.sync_info.on_wait = [w]
    drain.sync_info.on_wait = keep


def _build_nc(K1: int, K2: int):
    """Build the single-core graph (identical on all 8 cores)."""
    W = 2 * K2 + 66
    nc = bass.Bass()
    pk_ext = nc.declare_dram_parameter("pk", [128, W], F32, isOutput=False)
    out_ext = nc.declare_dram_parameter("out", [128, WIN * 128], F32, isOutput=True)

    c_dur, c_tok = 0, K2
    c_so = 2 * K2
    c_lc, c_lm = 2 * K2 + 1, 2 * K2 + 17
    c_o12, c_w12 = 2 * K2 + 33, 2 * K2 + 45
    c_o3, c_w3 = 2 * K2 + 57, 2 * K2 + 60
    c_bias = 2 * K2 + 63  # bias consts: [-1.25, -1.0, -21.0]

    with TileContext(nc) as tc:
        with tc.tile_pool(name="p", bufs=1) as pool:
            pk = pool.tile([128, W], F32, tag="pk")
            nc.sync.dma_start(pk[:], pk_ext[:])

            dur = pk[:, c_dur:c_dur + K2]
            tokg = pk[:, c_tok:c_tok + K2]
            so = pk[:, c_so:c_so + 1]
            lc = pk[:, c_lc:c_lc + WIN]
            lm = pk[:, c_lm:c_lm + WIN]

            # ---- rerelu(tok_g): [128, K2, 12] = clamp(tok - b_k, 0, .5) * c_k
            t144 = pool.tile([128, K2 * 12], F32, tag="t144")
            t144v = _vw(t144[:], [[12, K2], [1, 12]])
            tok_b = _vw(tokg, [[1, K2], [0, 12]])
            o12_b = _vw(pk[:, c_o12:c_o12 + 12], [[0, K2], [1, 12]])
            w12_b = _vw(pk[:, c_w12:c_w12 + 12], [[0, K2], [1, 12]])
            nc.vector.tensor_tensor(t144v, tok_b, o12_b, ALU.subtract)
            nc.vector.tensor_scalar(t144[:], t144[:], 0.0, 0.5, ALU.max, ALU.min)
            nc.vector.tensor_tensor(t144v, t144v, w12_b, ALU.mult)
            tok = pool.tile([128, K2], F32, tag="tok")
            nc.vector.tensor_reduce(tok[:], t144v, mybir.AxisListType.X, ALU.add)

            # ---- sum_token, rerelu2, new_dur
            st = pool.tile([128, K1], F32, tag="st")
            nc.vector.tensor_tensor(st[:], tok[:, 0:K1], tok[:, 1:K1 + 1], ALU.add)
            t33 = pool.tile([128, K1 * 3], F32, tag="t33")
            t33v = _vw(t33[:], [[3, K1], [1, 3]])
            st_b = _vw(st[:], [[1, K1], [0, 3]])
            o3_b = _vw(pk[:, c_o3:c_o3 + 3], [[0, K1], [1, 3]])
            w3_b = _vw(pk[:, c_w3:c_w3 + 3], [[0, K1], [1, 3]])
            nc.vector.tensor_tensor(t33v, st_b, o3_b, ALU.subtract)
            nc.vector.tensor_scalar(t33[:], t33[:], 0.0, 0.5, ALU.max, ALU.min)
            nc.vector.tensor_tensor(t33v, t33v, w3_b, ALU.mult)
            r2 = pool.tile([128, K1], F32, tag="r2")
            nc.vector.tensor_reduce(r2[:], t33v, mybir.AxisListType.X, ALU.add)
            nc.vector.tensor_scalar(r2[:], r2[:], 1.0, None, ALU.add)
            nd = pool.tile([128, K1], F32, tag="nd")
            nc.vector.tensor_tensor(nd[:], r2[:], dur[:, 1:K1 + 1], ALU.mult)
            nc.vector.tensor_tensor(nd[:], nd[:], dur[:, 0:K1], ALU.add)

            # ---- s (exclusive cumsum with start offset), ndc
            sbuf = pool.tile([128, K1 + 1], F32, tag="sbuf")
            nc.vector.tensor_copy(sbuf[:, 0:1], so)
            nc.vector.tensor_tensor_scan(
                sbuf[:, 1:K1 + 1], dur[:, 0:K1], dur[:, 0:K1], so,
                ALU.add, ALU.bypass)
            ndc = pool.tile([128, K1], F32, tag="ndc")
            nc.vector.tensor_tensor(ndc[:], sbuf[:, 0:K1], nd[:], ALU.add)

            # ---- rerelu3 -> a/4
            a4 = pool.tile([128, K1], F32, tag="a4")
            nc.vector.tensor_scalar(a4[:], tok[:, 0:K1], 1.25, 0.0,
                                    ALU.subtract, ALU.max)
            nc.vector.tensor_scalar(a4[:], a4[:], 0.5, None, ALU.min)
            nc.vector.tensor_tensor(a4[:], a4[:], a4[:], ALU.mult)

            # ---- cell stage [128, WIN, K1]
            FC = WIN * K1
            cells = pool.tile([128, FC], F32, tag="cells")
            cellsv = _vw(cells[:], [[K1, WIN], [1, K1]])
            m2 = pool.tile([128, FC], F32, tag="m2")
            m2v = _vw(m2[:], [[K1, WIN], [1, K1]])
            lc_b = _vw(lc, [[1, WIN], [0, K1]])
            lm_b = _vw(lm, [[1, WIN], [0, K1]])
            s_b = _vw(sbuf[:, 0:K1], [[0, WIN], [1, K1]])
            ndc_b = _vw(ndc[:], [[0, WIN], [1, K1]])
            a4_b = _vw(a4[:], [[0, WIN], [1, K1]])
            nc.vector.tensor_tensor(m2v, ndc_b, lm_b, ALU.is_gt)       # l < ndc
            nc.vector.tensor_tensor(cellsv, lc_b, s_b, ALU.subtract)   # l+1-s
            nc.vector.tensor_scalar(cells[:], cells[:], 0.0, None, ALU.max)
            nc.vector.tensor_tensor(cells[:], cells[:], m2[:], ALU.mult)
            nc.vector.tensor_tensor(cellsv, cellsv, a4_b, ALU.mult)
            y = pool.tile([128, WIN], F32, tag="y")
            nc.vector.tensor_reduce(y[:], cellsv, mybir.AxisListType.X, ALU.add)

            # ---- rerelu4(4y) = 32y - 33.2 relu(y-0.25) + 1.2 relu(y-5.25)
            q1 = pool.tile([128, WIN], F32, tag="q1")
            q21 = pool.tile([128, WIN], F32, tag="q21")
            nc.vector.tensor_scalar(q1[:], y[:], 0.25, 0.0, ALU.subtract, ALU.max)
            nc.vector.tensor_scalar(q21[:], y[:], 5.25, 0.0, ALU.subtract, ALU.max)
            mel = pool.tile([128, WIN], F32, tag="mel")
            nc.vector.tensor_scalar(mel[:], y[:], 32.0, None, ALU.mult)
            nc.vector.scalar_tensor_tensor(mel[:], q1[:], -33.2, mel[:],
                                           ALU.mult, ALU.add)
            nc.vector.scalar_tensor_tensor(mel[:], q21[:], 1.2, mel[:],
                                           ALU.mult, ALU.add)

            # ---- broadcast to 128 channels + store (chunked for DMA overlap)
            NCH = 1
            RPC = WIN // NCH
            for c in range(NCH):
                ot = pool.tile([128, RPC * 128], F32, tag=f"ot{c}")
                otv = _vw(ot[:], [[128, RPC], [1, 128]])
                mel_b = _vw(mel[:, c * RPC:(c + 1) * RPC], [[1, RPC], [0, 128]])
                nc.vector.tensor_copy(otv, mel_b)
                nc.sync.dma_start(
                    out_ext[:, c * RPC * 128:(c + 1) * RPC * 128], ot[:])
    _fix_drain_waits(nc)
    return nc


_CACHE = {}


def _host_prep(duration, token):
    dur = np.asarray(duration)[0].astype(np.int64)
    tokv = np.asarray(token)[0].astype(np.float32)
    T = dur.shape[0]
    s = np.concatenate([[0], np.cumsum(dur)])
    L = int(s[-1])
    assert L <= NWIN * WIN, f"L={L} exceeds {NWIN * WIN}"
    wstart = np.arange(NWIN, dtype=np.int64) * WIN
    f_w = np.maximum(np.searchsorted(s, wstart, side="right") - 2, 0)
    g_w = np.minimum(np.searchsorted(s, wstart + WIN, side="left"), T)
    Kmax = int(np.maximum(g_w - f_w, 0).max())
    K1 = Kmax + 1
    K2 = K1 + 1
    idx = f_w[:, None] + np.arange(K2)[None, :]
    valid = idx < T
    idxc = np.minimum(idx, T - 1)
    dur_g = np.where(valid, dur[idxc], 0).astype(np.float32)
    tok_g = np.where(valid, tokv[idxc], 0.0).astype(np.float32)
    start_off = s[f_w].astype(np.float32)[:, None]
    lconst = (wstart[:, None] + np.arange(WIN)[None, :] + 1).astype(np.float32)
    lm1 = lconst - 1.0

    W = 2 * K2 + 66
    pk = np.zeros((NWIN, W), dtype=np.float32)
    pk[:, 0:K2] = dur_g
    pk[:, K2:2 * K2] = tok_g
    pk[:, 2 * K2:2 * K2 + 1] = start_off
    pk[:, 2 * K2 + 1:2 * K2 + 17] = lconst
    pk[:, 2 * K2 + 17:2 * K2 + 33] = lm1
    pk[:, 2 * K2 + 33:2 * K2 + 45] = np.array([b for b, _ in PAIRS1], np.float32)
    pk[:, 2 * K2 + 45:2 * K2 + 57] = np.array([c for _, c in PAIRS1], np.float32)
    pk[:, 2 * K2 + 57:2 * K2 + 60] = np.array([b for b, _ in PAIRS2], np.float32)
    pk[:, 2 * K2 + 60:2 * K2 + 63] = np.array([c for _, c in PAIRS2], np.float32)
    pk[:, 2 * K2 + 63:2 * K2 + 66] = np.array([-1.25, -1.0, -21.0], np.float32)
    return pk, L, K1, K2


def _install_ntff_hook():
    """Provide the antenv.axon_hooks shim this image lacks, driving NTFF
    profiling via ctypes into libaxon_pjrt.so (mirrors trn_boot.py)."""
    import sys, types, ctypes, contextlib
    try:
        from antenv.axon_hooks import get_axon_ntff_profile_hook  # noqa
        return True
    except ImportError:
        pass
    so_path = "/opt/axon/libaxon_pjrt.so"
    try:
        lib = ctypes.CDLL(so_path)
    except OSError:
        return False
    if not hasattr(lib, "axon_start_nrt_profile"):
        return False
    lib.axon_start_nrt_profile.argtypes = [ctypes.POINTER(ctypes.c_int64),
                                           ctypes.c_size_t]
    lib.axon_start_nrt_profile.restype = ctypes.c_int64
    lib.axon_stop_nrt_profile.argtypes = [ctypes.c_char_p]
    lib.axon_stop_nrt_profile.restype = ctypes.c_int64

    @contextlib.contextmanager
    def _hook(output_dir, device_ids):
        import jax
        jax.devices()
        if device_ids:
            ids = (ctypes.c_int64 * len(device_ids))(*device_ids)
            rc = lib.axon_start_nrt_profile(ids, len(device_ids))
        else:
            rc = lib.axon_start_nrt_profile(None, 0)
        if rc != 0:
            raise RuntimeError(f"axon_start_nrt_profile rc={rc}")
        try:
            yield
        finally:
            n = lib.axon_stop_nrt_profile(str(output_dir).encode())
            print(f"ntff profile: {n} file(s) written to {output_dir}")

    mod = types.ModuleType("antenv.axon_hooks")
    mod.get_axon_ntff_profile_hook = lambda: _hook
    mod.set_axon_ntff_profile_hook = lambda h: None
    sys.modules["antenv.axon_hooks"] = mod
    return True


def kernel(duration, token, f0=None, _want_profile=False, **_ignored):
    if _want_profile:
        _install_ntff_hook()
    pk, L, K1, K2 = _host_prep(duration, token)
    key = (K1, K2)
    if key not in _CACHE:
        _CACHE[key] = _build_nc(K1, K2)
    nc = _CACHE[key]
    in_maps = [{"pk": pk[i * 128:(i + 1) * 128]} for i in range(N_CORES)]
    res = run_bass_kernel_spmd(nc, in_maps, list(range(N_CORES)),
                               trace=_want_profile)
    outs = [np.asarray(res.results[i]["out"]).reshape(128 * WIN, 128)
            for i in range(N_CORES)]
    mel_full = np.concatenate(outs, axis=0)[:L]
    out = mel_full[None, :, :].astype(np.float32)
    if _want_profile:
        return out, res
    return out
